# revision 3
# baseline (speedup 1.0000x reference)
"""RNN-T JointNetwork kernel for 8 Trainium2 NeuronCores.

Math: out[b,t,u,:] = tanh(concat(fe[b,t], gd[b,u])) @ Wj + bj
with fe = f@We+be, gd = g@Wd+bd.

Since tanh acts elementwise and the concat feeds a single GEMM, the joint
GEMM factorizes exactly:
    out[b,t,u,:] = A[b,t,:] + C[b,u,:]
    A = tanh(f@We+be) @ Wj[:Dm]          (per-(b,t) row)
    C = tanh(g@Wd+bd) @ Wj[Dm:] + bj     (per-(b,u) row)
This collapses the 137-GFLOP joint GEMM into two tiny GEMMs plus a
broadcast-add, leaving the kernel bound by the output write.

Sharding: 8 cores, core c owns (b = c//2, t-half = c%2) -> a [128,64,V]
output chunk per core.

Trace-driven design (profiled on trn2; ~72 us baseline -> this version):
  - DMA packets on one HWDGE queue drain near-FIFO per engine, so issue
    order IS priority order: pack1 (gates all compute) first, then wjb
    (C-path), wjt (A-path), sel.  Inputs are consolidated to 5 DMAs
    (biases ride inside pack1 as bf16; bj + a ones-row in a tiny [1,*]
    tensor) because each dma_start costs ~0.65 us of serial issue time
    on the Sync sequencer.
  - Both ACp tiles use the SAME layout [A-half ; C] so ONE selector
    serves all 16 chunks (saves the 512 KB sel1 load + the swap-identity
    matmuls of the previous design).  The A halves land in partitions
    0:64 of separate psum tiles via column-sliced stationary operands;
    C is computed once at partitions 64:128 and copied into both tiles.
  - A-half0 is computed before A-half1: chunks 0-7 need only ACp0, so
    the first output DMA issues ~6 us earlier.
  - psO->SBUF copies (the per-chunk [128,1024] fp32->bf16 moves) rotate
    across THREE engines (ACT/DVE/Pool) - at two engines they tied the
    DMA for steady-state bottleneck (~2.3 us/chunk each).
  - PE HAM clock gate starts at ~1.2 GHz; 8 dummy matmuls during the
    input DMA window warm it up (20 were blocking the PE queue past the
    pack1 arrival).
  - output rows are permuted so each partition writes 4 consecutive
    DRAM rows = one 8 KB descriptor (out tensor is bf16: tolerance 2e-2
    dwarfs bf16's ~5e-3; host upcasts to fp32).

On-core plan (bf16 everywhere, fp32 only in PSUM):
  - tfT[m,t] = tanh(We.T@fT + be), tgT likewise (PE bf16 + ACT tanh)
  - ACp0 = [A(0:64) ; C], ACp1 = [A(64:128) ; C] packed bf16 [128,V]
  - output chunk j covers out rows 512j..512j+512; psum tile a holds
    rows 4p+a so partition p's SBUF bytes map to 4 consecutive DRAM
    rows; ONE K=128 selector matmul per 512-col bank picks the A row
    and C row and sums them in fp32 PSUM
"""

import sys

sys.path.insert(0, "/opt/trn_rl_repo")

import numpy as np

import concourse.bacc as bacc
import concourse.mybir as mybir
import concourse.tile as tile
from concourse.bass_utils import run_bass_kernel_spmd

B, T, U = 4, 256, 64
D = 512  # DE = DD = DM
V = 1024
TC = 128  # t rows per core
NCORES = 8
FP32 = mybir.dt.float32
BF16 = mybir.dt.bfloat16
NPBF16 = mybir.dt.np(mybir.dt.bfloat16)
FP8 = mybir.dt.float8e4
NPFP8 = mybir.dt.np(mybir.dt.float8e4)
TANH = mybir.ActivationFunctionType.Tanh

# pack1 column offsets (per-core tensor: fT | We | gT | Wd | biases)
OFF_FT, OFF_WE, OFF_GT, OFF_WD, OFF_B = 0, 512, 2560, 2816, 4864
PACK1_COLS = 4872

_cache = {}


def _build_nc():
    nc = bacc.Bacc("TRN2", target_bir_lowering=False)

    pack1_d = nc.dram_tensor("pack1", [128, PACK1_COLS], BF16, kind="ExternalInput")
    brow_d = nc.dram_tensor("brow", [1, V + 64], BF16, kind="ExternalInput")
    wjb_d = nc.dram_tensor("wjb", [128, 4096], BF16, kind="ExternalInput")
    wjt_d = nc.dram_tensor("wjt", [128, 4096], BF16, kind="ExternalInput")
    sel_d = nc.dram_tensor("sel", [128, 4096], FP8, kind="ExternalInput")
    out_d = nc.dram_tensor("out", [16 * 128, 4 * V], BF16, kind="ExternalOutput")

    with tile.TileContext(nc) as tc:
        with tc.tile_pool(name="wts", bufs=1) as wp:
            pack1 = wp.tile([128, PACK1_COLS], BF16, tag="pack1")
            brow = wp.tile([1, V + 64], BF16, tag="brow")
            wjb = wp.tile([128, 4096], BF16, tag="wjb")
            wjt = wp.tile([128, 4096], BF16, tag="wjt")
            sel = wp.tile([128, 4096], FP8, tag="sel")
            tfT = [wp.tile([128, TC], BF16, tag=f"tfT{c}", name=f"tfT{c}") for c in range(4)]
            tgT = [wp.tile([128, U], BF16, tag=f"tgT{c}", name=f"tgT{c}") for c in range(4)]
            ACp0 = wp.tile([128, V], BF16, tag="ACp0")
            ACp1 = wp.tile([128, V], BF16, tag="ACp1")

            # input stream: one queue drains near-FIFO, so this order is
            # the arrival priority (see module doc)
            nc.sync.dma_start(pack1[:], pack1_d[:])
            nc.sync.dma_start(brow[:], brow_d[:])
            nc.sync.dma_start(wjb[:], wjb_d[:])
            nc.sync.dma_start(wjt[:], wjt_d[:])
            nc.sync.dma_start(sel[:], sel_d[:])

            # views into pack1
            fT = [pack1[:, OFF_FT + c * 128 : OFF_FT + (c + 1) * 128] for c in range(4)]
            We = [pack1[:, OFF_WE + c * 512 : OFF_WE + (c + 1) * 512] for c in range(4)]
            gT = [pack1[:, OFF_GT + c * 64 : OFF_GT + (c + 1) * 64] for c in range(4)]
            Wd = [pack1[:, OFF_WD + c * 512 : OFF_WD + (c + 1) * 512] for c in range(4)]
            be = lambda mc: pack1[:, OFF_B + mc : OFF_B + mc + 1]
            bd = lambda mc: pack1[:, OFF_B + 4 + mc : OFF_B + 5 + mc]
            # wj chunk mc, v-half vh
            wj_t = lambda mc, vh: wjt[:, 1024 * mc + 512 * vh : 1024 * mc + 512 * vh + 512]
            wj_b = lambda mc, vh: wjb[:, 1024 * mc + 512 * vh : 1024 * mc + 512 * vh + 512]
            ones64 = brow[:, V : V + 64]
            bj = lambda vh: brow[:, vh * 512 : (vh + 1) * 512]

            # ---- prologue ----
            with tc.tile_pool(name="pp", bufs=1, space="PSUM") as pp:
                # PE warm-up (see module doc); results never read
                scratch = wp.tile([128, 640], BF16, tag="scratch")
                nc.vector.memset(scratch[:], 1.0)
                wps = pp.tile([128, 512], FP32, tag="pps", bufs=3)
                for _ in range(8):
                    nc.tensor.matmul(
                        wps[:], scratch[:, 0:128], scratch[:, 128:640],
                        start=True, stop=True,
                    )

                # g-path first: C's dependency chain is longer than A's
                for mc in range(4):
                    ms = slice(mc * 128, (mc + 1) * 128)
                    ps = pp.tile([128, U], FP32, tag="pps", bufs=3)
                    for dc in range(4):
                        nc.tensor.matmul(
                            ps[:], Wd[dc][:, ms], gT[dc],
                            start=(dc == 0), stop=(dc == 3),
                        )
                    nc.scalar.activation(tgT[mc][:], ps[:], TANH, bias=bd(mc))
                for mc in range(4):
                    ms = slice(mc * 128, (mc + 1) * 128)
                    ps = pp.tile([128, TC], FP32, tag="pps", bufs=3)
                    for dc in range(4):
                        nc.tensor.matmul(
                            ps[:], We[dc][:, ms], fT[dc],
                            start=(dc == 0), stop=(dc == 3),
                        )
                    nc.scalar.activation(tfT[mc][:], ps[:], TANH, bias=be(mc))

                # C once at partitions 64:128 (+bj via K=1 ones matmul),
                # copied into BOTH ACp tiles (same partitions -> one sel)
                psC = []
                for vh in range(2):
                    ps = pp.tile([128, 512], FP32, tag="pj", bufs=5)
                    for mc in range(4):
                        nc.tensor.matmul(
                            ps[64:128, :], tgT[mc][:], wj_b(mc, vh),
                            start=(mc == 0), stop=False,
                        )
                    nc.tensor.matmul(
                        ps[64:128, :], ones64, bj(vh),
                        start=False, stop=True,
                    )
                    psC.append(ps)
                for vh in range(2):
                    vs = slice(vh * 512, (vh + 1) * 512)
                    nc.scalar.copy(ACp0[64:128, vs], psC[vh][64:128, :])
                    nc.vector.tensor_copy(ACp1[64:128, vs], psC[vh][64:128, :])

                # A-half h lands at partitions 0:64 via column-sliced
                # stationary operand; half0 first (chunks 0-7 need it)
                for h in range(2):
                    hs = slice(h * 64, (h + 1) * 64)
                    acp = (ACp0, ACp1)[h]
                    for vh in range(2):
                        vs = slice(vh * 512, (vh + 1) * 512)
                        ps = pp.tile([128, 512], FP32, tag="pj", bufs=5)
                        for mc in range(4):
                            nc.tensor.matmul(
                                ps[0:64, :], tfT[mc][:, hs], wj_t(mc, vh),
                                start=(mc == 0), stop=(mc == 3),
                            )
                        if h == 0:
                            nc.scalar.copy(acp[0:64, vs], ps[0:64, :])
                        else:
                            nc.vector.tensor_copy(acp[0:64, vs], ps[0:64, :])

            # ---- main loop: 16 chunks of [512 rows, 1024] bf16 = 1 MB ----
            # chunk j, psum tile a: psO_a[p,:] = out row 512j + 4p + a
            #   -> t = 8j + p//16, u = 4*(p%16) + a
            with (
                tc.tile_pool(name="po", bufs=4, space="PSUM") as po,
                tc.tile_pool(name="ob", bufs=4) as ob,
            ):
                for j in range(16):
                    h, jj = j // 8, j % 8
                    acp = (ACp0, ACp1)[h]
                    out_sb = ob.tile([128, 4 * V], BF16, tag="out")
                    for a in range(4):
                        psO = po.tile([128, V], FP32, tag="psO")
                        c0 = 128 * (4 * jj + a)
                        for vh in range(2):
                            nc.tensor.matmul(
                                psO[:, vh * 512 : (vh + 1) * 512],
                                sel[:, c0 : c0 + 128],
                                acp[:, vh * 512 : (vh + 1) * 512],
                                start=True, stop=True,
                            )
                        # PSUM->SBUF moves: only ACT/DVE can read PSUM on
                        # trn2 (GpSimd raises in the BIR verifier); split
                        # 2/2 per chunk, alternating which engine feeds
                        # the early half-DMA
                        dst = out_sb[:, a * V : (a + 1) * V]
                        on_act = (a + j) % 2 == 0
                        if on_act:
                            nc.scalar.copy(dst, psO[:])
                        else:
                            nc.vector.tensor_copy(dst, psO[:])
                        if j in (0, 15) and a == 1:
                            # first/last chunk ship in column halves: the
                            # first write starts two moves earlier and the
                            # final drain tail shrinks; the per-partition
                            # DRAM runs are still 4 KB
                            nc.sync.dma_start(
                                out_d[128 * j : 128 * (j + 1), 0 : 2 * V],
                                out_sb[:, 0 : 2 * V],
                            )
                    if j in (0, 15):
                        nc.sync.dma_start(
                            out_d[128 * j : 128 * (j + 1), 2 * V : 4 * V],
                            out_sb[:, 2 * V : 4 * V],
                        )
                    else:
                        nc.sync.dma_start(
                            out_d[128 * j : 128 * (j + 1), :], out_sb[:]
                        )

    nc.compile()
    return nc


def _chunkcat(M):
    """[N*128, C] -> [128, N*C]: stack 128-row chunks side by side."""
    n = M.shape[0] // 128
    return np.ascontiguousarray(
        M.reshape(n, 128, M.shape[1]).transpose(1, 0, 2).reshape(128, -1)
    )


def _build_selector():
    """Row-permuted pair selector (see main-loop comment)."""
    sel = np.zeros((128, 4096), np.float32)
    p = np.arange(128)
    for jj in range(8):
        for a in range(4):
            col = 128 * (4 * jj + a) + p
            tih = 8 * jj + p // 16
            u = 4 * (p % 16) + a
            sel[tih, col] = 1.0
            sel[64 + u, col] = 1.0
    return sel.astype(NPFP8)


def kernel(f, g, We, be, Wd, bd, Wj, bj):
    if "nc" not in _cache:
        _cache["nc"] = _build_nc()
    nc = _cache["nc"]

    b16 = lambda x: np.asarray(x, dtype=np.float32).astype(NPBF16)
    f = np.asarray(f, dtype=np.float32)
    g = np.asarray(g, dtype=np.float32)
    Wj = np.asarray(Wj, dtype=np.float32)

    sel = _build_selector()
    wjt = _chunkcat(b16(Wj[:D]))
    wjb = _chunkcat(b16(Wj[D:]))
    brow = np.zeros((1, V + 64), np.float32)
    brow[0, :V] = np.asarray(bj, dtype=np.float32)
    brow[0, V:] = 1.0
    bias8 = np.zeros((128, 8), np.float32)
    for c in range(4):
        bias8[:, c] = np.asarray(be, dtype=np.float32)[c * 128 : (c + 1) * 128]
        bias8[:, 4 + c] = np.asarray(bd, dtype=np.float32)[c * 128 : (c + 1) * 128]
    We_p = _chunkcat(b16(We))
    Wd_p = _chunkcat(b16(Wd))

    shared = {
        "wjt": wjt, "wjb": wjb, "sel": sel, "brow": b16(brow),
    }
    in_maps = []
    for c in range(NCORES):
        b, th = c // 2, c % 2
        fTp = _chunkcat(b16(f[b, th * TC : (th + 1) * TC, :].T))
        gTp = _chunkcat(b16(g[b].T))
        pack1 = np.concatenate([fTp, We_p, gTp, Wd_p, b16(bias8)], axis=1)
        in_maps.append({"pack1": np.ascontiguousarray(pack1), **shared})
    res = run_bass_kernel_spmd(nc, in_maps, list(range(NCORES)))
    kernel._last_results = res

    out = np.empty((B, T, U, V), np.float32)
    for c in range(NCORES):
        b, th = c // 2, c % 2
        out[b, th * TC : (th + 1) * TC] = (
            res.results[c]["out"].astype(np.float32).reshape(TC, U, V)
        )
    return out
